# revision 23
# baseline (speedup 1.0000x reference)
"""Trainium2 Bass kernel for nn_CSGO_model (4-layer transformer + 26 MLP heads).

Sharding: data-parallel over batch (8 batches/core) for the transformer;
tiny bf16 AllGather of x_comb; head-parallel (4 padded head slots/core, 32
slots for 26 heads) for the InvDynamic head MLPs.

Layout: activations are kept feature-major X^T [D, M] on chip (D on
partitions in 128-chunks, M = 8 local batches x 32 timesteps = 256 tokens on
the free dim), so every GEMM is matmul(psum, lhsT=W_chunk, rhs=X_chunk) with
no transposes; V is computed token-major via activation-stationary matmuls
(lhsT = xln chunk, rhs = wv streamed 1024 wide). All GEMMs run in bf16 with
fp32 PSUM accumulation; the residual stream and softmax stay fp32.

Key optimizations over the v0 kernel:
- LN column sums via float32r matmuls (full bf16-rate at 256-col streams,
  vs 4 cycles/row for plain fp32); sum matmul pre-scaled by 1/D.
- LN rsqrt computed as Exp(-0.5*Ln(v+eps)) on the ACT engine: Ln/Exp share
  one activation table set with attention's Exp, so the per-layer act-table
  reloads (1.28us each, on the LN critical path) disappear.
- The additive rank-5 -800 attention mask matmuls are replaced by a 0/1
  mask multiply on the (otherwise idle) GPSIMD engine after exp.
- Weights are streamed in per-output-chunk tiles (contiguous in DRAM)
  through small multi-buffered pools, so weight DMA double-buffers across
  the whole layer instead of stalling at layer boundaries.
- Layer 4 computes only what x_comb needs: K/V over all tokens, but
  Q/attention/out-proj/FF only at tokens {0,1} per batch (16 of 256
  columns), eliminating ~90% of layer-4 GEMM work.
- Head MLPs run feature-major (w1 chunks as lhsT) so no PE transposes and
  no cross-engine serialization between head layers.
- Dummy keep-warm matmuls cover the LN stats latency so the PE array's
  DVFS stays at full clock.
"""
import sys
import os
import types

sys.path.insert(0, '/opt/trn_rl_repo')

# bass_utils imports antenv.axon_hooks when BASS_TRACE is set; that module
# does not exist in this image, so install a no-op shim defensively.
if 'antenv.axon_hooks' not in sys.modules:
    try:
        from antenv import axon_hooks  # noqa: F401
    except ImportError:
        _hookmod = types.ModuleType('antenv.axon_hooks')
        _hookmod.set_axon_ntff_profile_hook = lambda h: None
        _hookmod.get_axon_ntff_profile_hook = lambda: None
        sys.modules['antenv.axon_hooks'] = _hookmod

import numpy as np
import ml_dtypes

BF16 = ml_dtypes.bfloat16

# Model dims
D = 1024
NHEADS = 16
HD = 64
INNER = NHEADS * HD
FF = 2048
L = 4
NOUT = 26
IDH = 512
B = 64
T = 32

N_CORES = 8
B_LOC = B // N_CORES          # 8 batches per core
M = B_LOC * T                 # 256 tokens per core
M4 = 2 * B_LOC                # 16 token columns used in layer 4 (t in {0,1})
DCH = D // 128                # 8 feature chunks
FCH = FF // 128               # 16
H_SLOTS = 4                   # padded head slots per core (8*4=32 >= 26)

N_WARM = 10                   # keep-warm dummy matmuls per LN site

_CACHE = {}


# ---------------------------------------------------------------- device code

def _build_nc():
    import concourse.tile as tile
    from concourse import mybir, bacc

    f32 = mybir.dt.float32
    f32r = mybir.dt.float32r
    bf16 = mybir.dt.bfloat16
    Alu = mybir.AluOpType
    Act = mybir.ActivationFunctionType

    nc = bacc.Bacc("TRN2", target_bir_lowering=False, debug=False,
                   num_devices=N_CORES)

    # ------------- DRAM tensors (per-core inputs, host-prepared layouts)
    x_d = nc.dram_tensor("x", [DCH, 128, M], f32, kind="ExternalInput")
    # weight streams: per-output-chunk tiles, contiguous per partition
    wqk_d = nc.dram_tensor("wqk", [L, 16, 128, DCH, 128], bf16,
                           kind="ExternalInput")
    wv_d = nc.dram_tensor("wv", [L, DCH, 128, INNER], bf16,
                          kind="ExternalInput")
    wo_d = nc.dram_tensor("wo", [L, DCH, 128, DCH, 128], bf16,
                          kind="ExternalInput")
    wf1_d = nc.dram_tensor("wf1", [L, FCH, 128, DCH, 128], bf16,
                           kind="ExternalInput")
    wf2_d = nc.dram_tensor("wf2", [L, DCH, 128, FCH, 128], bf16,
                           kind="ExternalInput")
    biasp_d = nc.dram_tensor("biasp", [128, L, 32], f32, kind="ExternalInput")
    qkbp_d = nc.dram_tensor("qkbp", [128, L, 2 * DCH], f32,
                            kind="ExternalInput")
    jones_d = nc.dram_tensor("jones", [128, 128], bf16, kind="ExternalInput")
    jones1_d = nc.dram_tensor("jones1", [128, 128], bf16,
                              kind="ExternalInput")
    mask01_d = nc.dram_tensor("mask01", [128, 2, M], bf16,
                              kind="ExternalInput")
    mask01t_d = nc.dram_tensor("mask01t", [128, 2, M4], bf16,
                               kind="ExternalInput")
    hw1_d = nc.dram_tensor("hw1", [H_SLOTS, 2 * DCH, 128, IDH], bf16,
                           kind="ExternalInput")
    hw2_d = nc.dram_tensor("hw2", [H_SLOTS, 4, 128, IDH], bf16,
                           kind="ExternalInput")
    hw3p_d = nc.dram_tensor("hw3p", [128, H_SLOTS * 4], bf16,
                            kind="ExternalInput")
    hb1p_d = nc.dram_tensor("hb1p", [128, H_SLOTS, 4], f32,
                            kind="ExternalInput")
    hb2p_d = nc.dram_tensor("hb2p", [128, H_SLOTS, 4], f32,
                            kind="ExternalInput")
    hb3p_d = nc.dram_tensor("hb3p", [1, H_SLOTS], f32, kind="ExternalInput")

    out_d = nc.dram_tensor("out_h", [1, H_SLOTS, B], f32,
                           kind="ExternalOutput")

    with tile.TileContext(nc) as tc:
        from contextlib import ExitStack
        with ExitStack() as ctx:
            const = ctx.enter_context(tc.tile_pool(name="const", bufs=1))
            ps_a = ctx.enter_context(
                tc.tile_pool(name="ps_a", bufs=5, space="PSUM"))
            ps_ln = ctx.enter_context(
                tc.tile_pool(name="ps_ln", bufs=1, space="PSUM"))
            ps_v = ctx.enter_context(
                tc.tile_pool(name="ps_v", bufs=1, space="PSUM"))
            dram = ctx.enter_context(
                tc.tile_pool(name="dram", bufs=1, space="DRAM"))
            # head pools outlive the transformer pools -> created first
            w1h_pool = ctx.enter_context(tc.tile_pool(name="w1h", bufs=4))
            w2h_pool = ctx.enter_context(tc.tile_pool(name="w2h", bufs=4))
            hact = ctx.enter_context(tc.tile_pool(name="hact", bufs=2))
            tfs = ctx.enter_context(ExitStack())
            hres = tfs.enter_context(tc.tile_pool(name="hres", bufs=1))
            sq = tfs.enter_context(tc.tile_pool(name="sq", bufs=4))
            stats = tfs.enter_context(tc.tile_pool(name="stats", bufs=6))
            actb = tfs.enter_context(tc.tile_pool(name="actb", bufs=2))
            qkp = tfs.enter_context(tc.tile_pool(name="qkp", bufs=1))
            vtokp = tfs.enter_context(tc.tile_pool(name="vtokp", bufs=1))
            attp = tfs.enter_context(tc.tile_pool(name="attp", bufs=6))
            obufp = tfs.enter_context(tc.tile_pool(name="obufp", bufs=1))
            g1p = tfs.enter_context(tc.tile_pool(name="g1p", bufs=1))
            wqk_pool = tfs.enter_context(tc.tile_pool(name="wqk", bufs=4))
            wv_pool = tfs.enter_context(tc.tile_pool(name="wv", bufs=3))
            wo_pool = tfs.enter_context(tc.tile_pool(name="wo", bufs=3))
            wf1_pool = tfs.enter_context(tc.tile_pool(name="wf1", bufs=4))
            wf2_pool = tfs.enter_context(tc.tile_pool(name="wf2", bufs=3))

            # constants
            jones = const.tile([128, 128], bf16, tag="jones")
            nc.sync.dma_start(jones[:], jones_d[:])
            jones1 = const.tile([128, 128], bf16, tag="jones1")
            nc.sync.dma_start(jones1[:], jones1_d[:])
            mask01 = const.tile([128, 2, M], bf16, tag="mask01")
            nc.sync.dma_start(mask01[:], mask01_d[:])
            mask01t = const.tile([128, 2, M4], bf16, tag="mask01t")
            nc.sync.dma_start(mask01t[:], mask01t_d[:])
            qkbp = const.tile([128, L, 2 * DCH], f32, tag="qkbp")
            nc.sync.dma_start(qkbp[:], qkbp_d[:])
            biasp = const.tile([128, L, 32], f32, tag="biasp")
            nc.sync.dma_start(biasp[:], biasp_d[:])
            # LN epsilons as [128,1] const APs (float act-bias needs an AP)
            eps0 = const.tile([128, 1], f32, tag="eps0")
            nc.vector.memset(eps0[:], 1e-6)
            eps1 = const.tile([128, 1], f32, tag="eps1")
            nc.vector.memset(eps1[:], 1e-5)

            # residual, feature-major [128p, chunk, token], fp32
            h = hres.tile([128, DCH, M], f32, tag="h")
            for c in range(DCH):
                nc.sync.dma_start(h[:, c, :], x_d[c])

            w1h_tiles, w2h_tiles = [], []

            def warm(n):
                """Keep-warm dummy matmuls: hold the PE busy (and its DVFS
                clock up) while the LN stats chain runs on DVE/ACT."""
                if n <= 0:
                    return
                ps_w = ps_a.tile([128, 2, M], f32, tag="a", name=None)
                for wi in range(n):
                    nc.tensor.matmul(ps_w[:, wi % 2, :], jones[:],
                                     mask01[:, 0, :],
                                     start=(wi < 2), stop=(wi >= n - 2))

            def layer_norm(l, site, src, mcols, mtag):
                """LN over the feature (partition*chunk) axis.

                src: [128, DCH, mcols] fp32 tile (or view).
                Returns xln [128, DCH, mcols] bf16.
                site 0: collapsed double-LN -> rsqrt(v + 1e-6), host folds
                the 1/sqrt(1+1e-5) into g.  site 1: rsqrt(v + 1e-5).
                alpha = Exp(-0.5 * Ln(v + eps)) (stays in the exp table set).
                """
                eps = eps0 if site == 0 else eps1
                # ps_sq[:,0,:] = mean, ps_sq[:,1,:] = E[h^2]; both via bf16
                # ones-matmuls (jones is pre-scaled by 1/D on the host).
                ps_sq = ps_ln.tile([128, 2, mcols], f32, tag="ln")
                for c in range(DCH):
                    hb = sq.tile([128, mcols], bf16, tag="sqc" + mtag)
                    nc.gpsimd.tensor_copy(hb[:], src[:, c, :])
                    nc.tensor.matmul(ps_sq[:, 0, :], jones[:], hb[:],
                                     start=(c == 0), stop=(c == DCH - 1))
                for c in range(DCH):
                    hsq = sq.tile([128, mcols], bf16, tag="sq" + mtag)
                    nc.gpsimd.tensor_mul(hsq[:], src[:, c, :], src[:, c, :])
                    nc.tensor.matmul(ps_sq[:, 1, :], jones[:], hsq[:],
                                     start=(c == 0), stop=(c == DCH - 1))
                warm(N_WARM)
                mu_t = stats.tile([128, mcols], f32, tag="st" + mtag)
                nc.vector.tensor_copy(mu_t[:], ps_sq[:, 0, :])
                mu = mu_t[:]
                mu2 = stats.tile([128, mcols], f32, tag="st" + mtag)
                nc.vector.tensor_mul(mu2[:], mu, mu)
                v = stats.tile([128, mcols], f32, tag="st" + mtag)
                nc.vector.tensor_sub(v[:], ps_sq[:, 1, :], mu2[:])
                lnv = stats.tile([128, mcols], f32, tag="st" + mtag)
                nc.scalar.activation(lnv[:], v[:], Act.Ln, bias=eps[:])
                alpha = stats.tile([128, mcols], f32, tag="st" + mtag)
                nc.scalar.activation(alpha[:], lnv[:], Act.Exp, scale=-0.5)
                xln = actb.tile([128, DCH, mcols], bf16, tag="xln" + mtag)
                for cs in ((0, 2), (2, 4), (4, 6), (6, 8)):
                    c0, c1 = cs
                    w = c1 - c0
                    tt_full = sq.tile([128, 2, mcols], f32, tag="sqf" + mtag)
                    t = tt_full[:, :w, :]
                    mu_b = mu.unsqueeze(1).to_broadcast([128, w, mcols])
                    al_b = alpha[:].unsqueeze(1).to_broadcast([128, w, mcols])
                    nc.vector.tensor_sub(t[:], src[:, c0:c1, :], mu_b)
                    nc.vector.tensor_mul(xln[:, c0:c1, :], t[:], al_b)
                return xln

            for l in range(L):
                last = (l == L - 1)
                # number of query-token columns this layer actually needs
                mq = M4 if last else M
                msk = mask01t if last else mask01

                # ---- attn pre-LN (collapsed double LN)
                xln = layer_norm(l, 0, h[:], M, "")

                # t01 columns of xln for layer-4 Q, packed [128, DCH, 16]
                if last:
                    xln_q = actb.tile([128, DCH, M4], bf16, tag="xlnq")
                    nc.gpsimd.tensor_copy(
                        xln_q[:],
                        xln[:].rearrange("p c (b t) -> p c b t",
                                         b=B_LOC)[:, :, :, 0:2])

                # ---- Q,K feature-major GEMM -> qk [128, 2*DCH, M]
                # (layer 4: Q only at t01 columns, stored in qk[:, n, 0:16])
                qk = qkp.tile([128, 2 * DCH, M], bf16, tag="qk")
                for np_ in range(DCH):
                    ps = ps_a.tile([128, 2, M], f32, tag="a")
                    for i in range(2):
                        n = 2 * np_ + i            # Q chunk index 0..7 x2
                        wt = wqk_pool.tile([128, DCH, 128], bf16, tag="wqk")
                        nc.sync.dma_start(wt[:], wqk_d[l, n])
                        if last and n < DCH:
                            for c in range(DCH):
                                nc.tensor.matmul(ps[:, i, 0:M4],
                                                 wt[:, c, :], xln_q[:, c, :],
                                                 start=(c == 0),
                                                 stop=(c == DCH - 1))
                        else:
                            for c in range(DCH):
                                nc.tensor.matmul(ps[:, i, :],
                                                 wt[:, c, :], xln[:, c, :],
                                                 start=(c == 0),
                                                 stop=(c == DCH - 1))
                    for i in range(2):
                        n = 2 * np_ + i
                        w_cols = M4 if (last and n < DCH) else M
                        nc.scalar.activation(qk[:, n, 0:w_cols],
                                             ps[:, i, 0:w_cols],
                                             Act.Identity,
                                             bias=qkbp[:, l, n:n + 1])

                # ---- V token-major GEMM (activation-stationary):
                # vtok [128 tok, 2, INNER]
                vtok = vtokp.tile([128, 2, INNER], bf16, tag="vtok")
                for mc in range(2):
                    psv = ps_v.tile([128, 2, 512], f32, tag="v")
                    for c in range(DCH):
                        wvt = wv_pool.tile([128, INNER], bf16, tag="wv")
                        nc.sync.dma_start(wvt[:], wv_d[l, c])
                        for jb in range(2):
                            nc.tensor.matmul(
                                psv[:, jb, :],
                                xln[:, c, mc * 128:(mc + 1) * 128],
                                wvt[:, jb * 512:(jb + 1) * 512],
                                start=(c == 0), stop=(c == DCH - 1))
                    nc.scalar.activation(vtok[:, mc, :], psv[:], Act.Copy)

                # ---- attention, head-pair by head-pair
                # (layer 4: only mq=16 query columns)
                obuf = obufp.tile([128, DCH, M], bf16, tag="obuf")
                for hc in range(DCH):
                    # head pair (2*hc, 2*hc+1): even at partitions 0:64,
                    # odd at 64:128 -> S matmuls interleave across row
                    # groups so the PE array runs them concurrently.
                    es = []
                    pss = []
                    for j in range(2):
                        e_j = attp.tile([128, 2, mq], bf16, tag="e",
                                        name=f"e_{l}_{hc}_{j}")
                        ps_j = ps_a.tile([128, 2, mq], f32, tag="a",
                                         name=f"pss_{l}_{hc}_{j}")
                        es.append(e_j)
                        pss.append(ps_j)
                    for i in range(2):
                        for j in range(2):
                            hp = j * 64
                            nc.tensor.matmul(
                                pss[j][:, i, :],
                                qk[hp:hp + 64, DCH + hc,
                                   i * 128:(i + 1) * 128],
                                qk[hp:hp + 64, hc, 0:mq],
                                start=True, stop=True)
                    for j in range(2):
                        nc.scalar.activation(es[j][:], pss[j][:], Act.Exp,
                                             scale=0.125)
                    # zero cross-batch pairs (block-diag mask) on GPSIMD
                    for j in range(2):
                        nc.gpsimd.tensor_mul(es[j][:], es[j][:], msk[:])
                    for j in range(2):
                        hh = 2 * hc + j
                        hp = j * 64
                        e = es[j]
                        ps_dn = ps_a.tile([128, mq], f32, tag="a")
                        for i in range(2):
                            nc.tensor.matmul(ps_dn[:], jones1[:], e[:, i, :],
                                             start=(i == 0), stop=(i == 1))
                        rd = stats.tile([128, mq], f32, tag="strd")
                        nc.vector.reciprocal_approx_fast(rd[0:64, :],
                                                         ps_dn[0:64, :])
                        ps_o = ps_a.tile([128, mq], f32, tag="a")
                        for i in range(2):
                            nc.tensor.matmul(
                                ps_o[hp:hp + 64, :],
                                vtok[:, i, hh * 64:(hh + 1) * 64],
                                e[:, i, :],
                                start=(i == 0), stop=(i == 1),
                                tile_position=(0, hp))
                        nc.vector.tensor_tensor(
                            obuf[hp:hp + 64, hc, 0:mq], ps_o[hp:hp + 64, :],
                            rd[0:64, :], Alu.mult)

                # ---- output projection (+ residual + out_b)
                # layer 4: only t01 columns, into h_mid (no h update needed)
                if last:
                    h_mid = hres.tile([128, DCH, M4], f32, tag="hmid")
                    h_t01 = h[:].rearrange("p c (b t) -> p c b t",
                                           b=B_LOC)[:, :, :, 0:2]
                for n in range(DCH):
                    wot = wo_pool.tile([128, DCH, 128], bf16, tag="wo")
                    nc.sync.dma_start(wot[:], wo_d[l, n])
                    ps = ps_a.tile([128, mq], f32, tag="a")
                    for c in range(DCH):
                        nc.tensor.matmul(ps[:],
                                         wot[:, c, :],
                                         obuf[:, c, 0:mq],
                                         start=(c == 0), stop=(c == DCH - 1))
                    if last:
                        nc.vector.scalar_tensor_tensor(
                            h_mid[:, n, :], ps[:], biasp[:, l, n:n + 1],
                            h_t01[:, n], Alu.add, Alu.add)
                    else:
                        nc.vector.scalar_tensor_tensor(
                            h[:, n, :], ps[:], biasp[:, l, n:n + 1],
                            h[:, n, :], Alu.add, Alu.add)

                # ---- ff pre-LN
                if last:
                    xln2 = layer_norm(l, 1, h_mid[:], M4, "t")
                else:
                    xln2 = layer_norm(l, 1, h[:], M, "")

                # ---- ff1 + gelu(x + b1) -> g1 [128, FCH, mq]
                g1 = g1p.tile([128, FCH, M], bf16, tag="g1")
                for n in range(FCH):
                    wft = wf1_pool.tile([128, DCH, 128], bf16, tag="wf1")
                    nc.sync.dma_start(wft[:], wf1_d[l, n])
                    ps = ps_a.tile([128, mq], f32, tag="a")
                    for c in range(DCH):
                        nc.tensor.matmul(ps[:],
                                         wft[:, c, :],
                                         xln2[:, c, :],
                                         start=(c == 0), stop=(c == DCH - 1))
                    nc.scalar.activation(g1[:, n, 0:mq], ps[:], Act.Gelu,
                                         bias=biasp[:, l, 8 + n:9 + n])

                # ---- ff2 (+ residual + b2); layer 4 writes x16 instead
                if last:
                    x16 = const.tile([128, DCH, 2, B_LOC], bf16, tag="x16")
                    hm_v = h_mid[:].rearrange("p c (b t) -> p c b t", b=B_LOC)
                for n in range(DCH):
                    wft = wf2_pool.tile([128, FCH, 128], bf16, tag="wf2")
                    nc.sync.dma_start(wft[:], wf2_d[l, n])
                    ps = ps_a.tile([128, mq], f32, tag="a")
                    for c in range(FCH):
                        nc.tensor.matmul(ps[:],
                                         wft[:, c, :],
                                         g1[:, c, 0:mq],
                                         start=(c == 0), stop=(c == FCH - 1))
                    if last:
                        # x16[:, n, tt, b] = ps[(b,t)] + b2 + h_mid[(b,t)]
                        nc.vector.scalar_tensor_tensor(
                            x16[:, n, :, :].rearrange("p tt b -> p b tt"),
                            ps[:], biasp[:, l, 24 + n:25 + n],
                            hm_v[:, n], Alu.add, Alu.add)
                    else:
                        nc.vector.scalar_tensor_tensor(
                            h[:, n, :], ps[:], biasp[:, l, 24 + n:25 + n],
                            h[:, n, :], Alu.add, Alu.add)

                # prefetch head weights during layers 2 and 3 (2 slots each)
                if l in (1, 2):
                    for sn in (2 * (l - 1), 2 * (l - 1) + 1):
                        w1h = w1h_pool.tile([128, 2 * DCH, IDH], bf16,
                                            tag="w1h")
                        nc.sync.dma_start(
                            w1h[:], hw1_d[sn].rearrange("c p n2 -> p c n2"))
                        w2h = w2h_pool.tile([128, 4, IDH], bf16, tag="w2h")
                        nc.sync.dma_start(
                            w2h[:], hw2_d[sn].rearrange("c p n2 -> p c n2"))
                        w1h_tiles.append(w1h)
                        w2h_tiles.append(w2h)

            # ---------------- AllGather x_comb ----------------
            tfs.close()   # free transformer pools for the heads stage
            cc_in = dram.tile([128, 128], bf16)
            nc.sync.dma_start(cc_in[:],
                              x16[:].rearrange("p c tt b -> p (c tt b)"))
            cc_out = dram.tile([N_CORES * 128, 128], bf16)
            nc.gpsimd.collective_compute(
                "AllGather", Alu.bypass,
                replica_groups=[list(range(N_CORES))],
                ins=[cc_in[:].opt()], outs=[cc_out[:].opt()])

            gsb = const.tile([128, N_CORES, 128], bf16, tag="gsb")
            nc.sync.dma_start(
                gsb[:], cc_out[:].rearrange("(j p) f -> p j f", p=128))
            # PE warm-up after the AllGather idle gap
            ps_w = ps_a.tile([128, 2, M], f32, tag="a", name="warmup_ps")
            for wi in range(12):
                nc.tensor.matmul(ps_w[:, wi % 2, :], jones[:],
                                 mask01[:, 0, :],
                                 start=(wi < 2), stop=(wi >= 10))

            # ---------------- 26 (padded 32) MLP heads, feature-major ------
            hb1p = const.tile([128, H_SLOTS, 4], f32, tag="hb1p")
            nc.sync.dma_start(hb1p[:], hb1p_d[:])
            hb2p = const.tile([128, H_SLOTS, 4], f32, tag="hb2p")
            nc.sync.dma_start(hb2p[:], hb2p_d[:])
            hb3p = const.tile([1, H_SLOTS], f32, tag="hb3p")
            nc.sync.dma_start(hb3p[:], hb3p_d[:])
            hw3 = const.tile([128, H_SLOTS * 4], bf16, tag="hw3")
            nc.sync.dma_start(hw3[:], hw3p_d[:])
            outacc = const.tile([1, H_SLOTS, B], f32, tag="outacc")

            # x_comb^T [2D, B] packed: kc = tt*DCH + c, batch = j*8 + b
            # (gsb free layout per core j is (c, tt, b)).
            xcombT = const.tile([128, 2 * DCH, B], bf16, tag="xcombT")
            for kc in range(2 * DCH):
                tt, c = kc // DCH, kc % DCH
                nc.vector.tensor_copy(
                    xcombT[:, kc, :],
                    gsb[:, :, c * 16 + tt * 8: c * 16 + tt * 8 + 8])

            def xcomb_rhs(kc):
                return xcombT[:, kc, :]

            for n in range(H_SLOTS):
                w1h = w1h_tiles[n]
                w2h = w2h_tiles[n]
                # h1^T [512, 64] feature-major, 4 idh chunks
                ps1 = ps_a.tile([128, 4, B], f32, tag="a")
                for nc2 in range(4):
                    for kc in range(2 * DCH):
                        nc.tensor.matmul(
                            ps1[:, nc2, :],
                            w1h[:, kc, nc2 * 128:(nc2 + 1) * 128],
                            xcomb_rhs(kc),
                            start=(kc == 0), stop=(kc == 2 * DCH - 1))
                h1t = hact.tile([128, 4, B], bf16, tag="h1t")
                for nc2 in range(4):
                    nc.scalar.activation(h1t[:, nc2, :], ps1[:, nc2, :],
                                         Act.Relu,
                                         bias=hb1p[:, n, nc2:nc2 + 1])
                ps2 = ps_a.tile([128, 4, B], f32, tag="a")
                for nc2 in range(4):
                    for kc in range(4):
                        nc.tensor.matmul(
                            ps2[:, nc2, :],
                            w2h[:, kc, nc2 * 128:(nc2 + 1) * 128],
                            h1t[:, kc, :],
                            start=(kc == 0), stop=(kc == 3))
                h2t = hact.tile([128, 4, B], bf16, tag="h2t")
                for nc2 in range(4):
                    nc.scalar.activation(h2t[:, nc2, :], ps2[:, nc2, :],
                                         Act.Relu,
                                         bias=hb2p[:, n, nc2:nc2 + 1])
                ps3 = ps_a.tile([1, B], f32, tag="a")
                for kc in range(4):
                    nc.tensor.matmul(ps3[:],
                                     hw3[:, n * 4 + kc:n * 4 + kc + 1],
                                     h2t[:, kc, :],
                                     start=(kc == 0), stop=(kc == 3))
                nc.vector.tensor_scalar_add(outacc[:, n, :], ps3[:],
                                            hb3p[0:1, n:n + 1])

            nc.sync.dma_start(out_d[:], outacc[:])

    nc.finalize()
    return nc


# ---------------------------------------------------------------- host side

def _prep_in_maps(inputs):
    x = np.asarray(inputs['x'], np.float32)
    qkv_w = np.asarray(inputs['qkv_w'], np.float32)
    out_w = np.asarray(inputs['out_w'], np.float32)
    out_b = np.asarray(inputs['out_b'], np.float32)
    attn_ln_g = np.asarray(inputs['attn_ln_g'], np.float32)
    attn_ln_b = np.asarray(inputs['attn_ln_b'], np.float32)
    ff_ln_g = np.asarray(inputs['ff_ln_g'], np.float32)
    ff_ln_b = np.asarray(inputs['ff_ln_b'], np.float32)
    ff_w1 = np.asarray(inputs['ff_w1'], np.float32)
    ff_b1 = np.asarray(inputs['ff_b1'], np.float32)
    ff_w2 = np.asarray(inputs['ff_w2'], np.float32)
    ff_b2 = np.asarray(inputs['ff_b2'], np.float32)
    head_w1 = np.asarray(inputs['head_w1'], np.float32)
    head_b1 = np.asarray(inputs['head_b1'], np.float32)
    head_w2 = np.asarray(inputs['head_w2'], np.float32)
    head_b2 = np.asarray(inputs['head_b2'], np.float32)
    head_w3 = np.asarray(inputs['head_w3'], np.float32)
    head_b3 = np.asarray(inputs['head_b3'], np.float32)

    # Fold the LN affine transform into the following GEMM weights (exact):
    #   xln = (h-mu)*alpha_hat;  y = (xln*g + b) @ W = xln @ (diag(g) W) + b@W
    # The collapsed double-LN 1/sqrt(1+1e-5) factor is folded into g too.
    ag_eff = attn_ln_g * np.float32((1.0 + 1e-5) ** -0.5)   # [L, D]
    qkvb = np.einsum('ld,ldn->ln', attn_ln_b, qkv_w)        # [L, 3*INNER]
    ff_b1 = ff_b1 + np.einsum('ld,ldn->ln', ff_ln_b, ff_w1)
    qkv_w = qkv_w * ag_eff[:, :, None]
    ff_w1 = ff_w1 * ff_ln_g[:, :, None]
    # V's LN-bias contribution passes through softmax unchanged (weights sum
    # to 1), so it folds into the output-projection bias exactly.
    vbias = qkvb[:, 2 * INNER:]                              # [L, INNER]
    out_b = out_b + np.einsum('lk,lkd->ld', vbias, out_w)

    # shared (replicated) weight tensors in per-chunk stream layouts
    wqk = np.zeros((L, 16, 128, DCH, 128), np.float32)
    wv = np.zeros((L, DCH, 128, INNER), np.float32)
    wo = np.zeros((L, DCH, 128, DCH, 128), np.float32)
    wf1 = np.zeros((L, FCH, 128, DCH, 128), np.float32)
    wf2 = np.zeros((L, DCH, 128, FCH, 128), np.float32)
    for l in range(L):
        wqk[l] = qkv_w[l][:, :2 * INNER].reshape(
            DCH, 128, 16, 128).transpose(2, 1, 0, 3)
        wv[l] = qkv_w[l][:, 2 * INNER:].reshape(DCH, 128, INNER)
        wo[l] = out_w[l].reshape(DCH, 128, DCH, 128).transpose(2, 1, 0, 3)
        wf1[l] = ff_w1[l].reshape(DCH, 128, FCH, 128).transpose(2, 1, 0, 3)
        wf2[l] = ff_w2[l].reshape(FCH, 128, DCH, 128).transpose(2, 1, 0, 3)
    wqk = np.ascontiguousarray(wqk).astype(BF16)
    wv = np.ascontiguousarray(wv).astype(BF16)
    wo = np.ascontiguousarray(wo).astype(BF16)
    wf1 = np.ascontiguousarray(wf1).astype(BF16)
    wf2 = np.ascontiguousarray(wf2).astype(BF16)

    biasp = np.zeros((128, L, 32), np.float32)
    biasp[:, :, 0:8] = out_b.reshape(L, 8, 128).transpose(2, 0, 1)
    biasp[:, :, 8:24] = ff_b1.reshape(L, 16, 128).transpose(2, 0, 1)
    biasp[:, :, 24:32] = ff_b2.reshape(L, 8, 128).transpose(2, 0, 1)

    qkbp = np.ascontiguousarray(
        qkvb[:, :2 * INNER].reshape(L, 2 * DCH, 128).transpose(2, 0, 1))

    jones = np.full((128, 128), 1.0 / D, np.float32).astype(BF16)
    jones1 = np.ones((128, 128), np.float32).astype(BF16)

    # 0/1 block-diagonal attention mask: key token k = i*128 + p (batch
    # k//32), query token m (batch m//32); full M and t01-only variants.
    mask01 = np.zeros((128, 2, M), np.float32)
    for i in range(2):
        for p in range(128):
            kb = (i * 128 + p) // T
            mask01[p, i, kb * T:(kb + 1) * T] = 1.0
    mask01t = np.zeros((128, 2, M4), np.float32)
    for i in range(2):
        for p in range(128):
            kb = (i * 128 + p) // T
            mask01t[p, i, kb * 2:(kb + 1) * 2] = 1.0
    mask01 = mask01.astype(BF16)
    mask01t = mask01t.astype(BF16)

    in_maps = []
    for c in range(N_CORES):
        xs = x[c * B_LOC:(c + 1) * B_LOC].reshape(M, D)  # [256, 1024]
        x_fm = np.ascontiguousarray(xs.T.reshape(DCH, 128, M))

        hw1 = np.zeros((H_SLOTS, 2 * DCH, 128, IDH), np.float32)
        hw2 = np.zeros((H_SLOTS, 4, 128, IDH), np.float32)
        hw3p = np.zeros((128, H_SLOTS * 4), np.float32)
        hb1p = np.zeros((128, H_SLOTS, 4), np.float32)
        hb2p = np.zeros((128, H_SLOTS, 4), np.float32)
        hb3p = np.zeros((1, H_SLOTS), np.float32)
        for n in range(H_SLOTS):
            g = n * N_CORES + c
            if g >= NOUT:
                continue
            hw1[n] = head_w1[g].reshape(2 * DCH, 128, IDH)
            hw2[n] = head_w2[g].reshape(4, 128, IDH)
            hw3p[:, n * 4:(n + 1) * 4] = head_w3[g].reshape(4, 128).T
            hb1p[:, n, :] = head_b1[g].reshape(4, 128).T
            hb2p[:, n, :] = head_b2[g].reshape(4, 128).T
            hb3p[0, n] = head_b3[g, 0]
        in_maps.append({
            'x': x_fm,
            'wqk': wqk, 'wv': wv, 'wo': wo, 'wf1': wf1, 'wf2': wf2,
            'biasp': biasp, 'qkbp': qkbp,
            'jones': jones, 'jones1': jones1,
            'mask01': mask01, 'mask01t': mask01t,
            'hw1': hw1.astype(BF16), 'hw2': hw2.astype(BF16),
            'hw3p': hw3p.astype(BF16),
            'hb1p': hb1p, 'hb2p': hb2p, 'hb3p': hb3p,
        })
    return in_maps


def _get_nc():
    if 'nc' not in _CACHE:
        _CACHE['nc'] = _build_nc()
    return _CACHE['nc']


def _unshard_out(results):
    out = np.zeros((B, NOUT, 1), np.float32)
    for c in range(N_CORES):
        oh = results[c]['out_h']           # [1, H_SLOTS, B]
        for n in range(H_SLOTS):
            g = n * N_CORES + c
            if g < NOUT:
                out[:, g, 0] = oh[0, n, :]
    return out


def kernel(**inputs):
    from concourse.bass_utils import run_bass_kernel_spmd
    nc = _get_nc()
    in_maps = _prep_in_maps(inputs)
    res = run_bass_kernel_spmd(nc, in_maps, core_ids=list(range(N_CORES)))
    return _unshard_out(res.results)


# revision 27
# speedup vs baseline: 1.0555x; 1.0555x over previous
"""Trainium2 Bass kernel for nn_CSGO_model (4-layer transformer + 26 MLP heads).

Sharding: data-parallel over batch (8 batches/core) for the transformer;
tiny bf16 AllGather of x_comb; head-parallel (4 padded head slots/core, 32
slots for 26 heads) for the InvDynamic head MLPs.

Layout: activations are kept feature-major X^T [D, M] on chip (D on
partitions in 128-chunks, M = 8 local batches x 32 timesteps = 256 tokens on
the free dim), so every GEMM is matmul(psum, lhsT=W_chunk, rhs=X_chunk) with
no transposes; V is computed token-major via activation-stationary matmuls
(lhsT = xln chunk, rhs = wv streamed 1024 wide). All GEMMs run in bf16 with
fp32 PSUM accumulation; the residual stream and softmax stay fp32.

Key optimizations over the v0 kernel:
- LN column sums via float32r matmuls (full bf16-rate at 256-col streams,
  vs 4 cycles/row for plain fp32); sum matmul pre-scaled by 1/D.
- LN rsqrt computed as Exp(-0.5*Ln(v+eps)) on the ACT engine: Ln/Exp share
  one activation table set with attention's Exp, so the per-layer act-table
  reloads (1.28us each, on the LN critical path) disappear.
- The additive rank-5 -800 attention mask matmuls are replaced by a 0/1
  mask multiply on the (otherwise idle) GPSIMD engine after exp.
- Weights are streamed in per-output-chunk tiles (contiguous in DRAM)
  through small multi-buffered pools, so weight DMA double-buffers across
  the whole layer instead of stalling at layer boundaries.
- Layer 4 computes only what x_comb needs: K/V over all tokens, but
  Q/attention/out-proj/FF only at tokens {0,1} per batch (16 of 256
  columns), eliminating ~90% of layer-4 GEMM work.
- Head MLPs run feature-major (w1 chunks as lhsT) so no PE transposes and
  no cross-engine serialization between head layers.
- Dummy keep-warm matmuls cover the LN stats latency so the PE array's
  DVFS stays at full clock.
"""
import sys
import os
import types

sys.path.insert(0, '/opt/trn_rl_repo')

# bass_utils imports antenv.axon_hooks when BASS_TRACE is set; that module
# does not exist in this image, so install a no-op shim defensively.
if 'antenv.axon_hooks' not in sys.modules:
    try:
        from antenv import axon_hooks  # noqa: F401
    except ImportError:
        _hookmod = types.ModuleType('antenv.axon_hooks')
        _hookmod.set_axon_ntff_profile_hook = lambda h: None
        _hookmod.get_axon_ntff_profile_hook = lambda: None
        sys.modules['antenv.axon_hooks'] = _hookmod

import numpy as np
import ml_dtypes

BF16 = ml_dtypes.bfloat16

# Model dims
D = 1024
NHEADS = 16
HD = 64
INNER = NHEADS * HD
FF = 2048
L = 4
NOUT = 26
IDH = 512
B = 64
T = 32

N_CORES = 8
B_LOC = B // N_CORES          # 8 batches per core
M = B_LOC * T                 # 256 tokens per core
M4 = 2 * B_LOC                # 16 token columns used in layer 4 (t in {0,1})
DCH = D // 128                # 8 feature chunks
FCH = FF // 128               # 16
H_SLOTS = 4                   # padded head slots per core (8*4=32 >= 26)

N_WARM = 10                   # keep-warm dummy matmuls per LN site

_CACHE = {}


# ---------------------------------------------------------------- device code

def _build_nc():
    import concourse.tile as tile
    from concourse import mybir, bacc

    f32 = mybir.dt.float32
    f32r = mybir.dt.float32r
    bf16 = mybir.dt.bfloat16
    Alu = mybir.AluOpType
    Act = mybir.ActivationFunctionType

    nc = bacc.Bacc("TRN2", target_bir_lowering=False, debug=False,
                   num_devices=N_CORES)

    # ------------- DRAM tensors (per-core inputs, host-prepared layouts)
    x_d = nc.dram_tensor("x", [DCH, 128, M], f32, kind="ExternalInput")
    # weight streams: per-output-chunk tiles, contiguous per partition
    wqk_d = nc.dram_tensor("wqk", [L, 16, 128, DCH, 128], bf16,
                           kind="ExternalInput")
    wv_d = nc.dram_tensor("wv", [L, DCH, 128, INNER], bf16,
                          kind="ExternalInput")
    wo_d = nc.dram_tensor("wo", [L, DCH, 128, DCH, 128], bf16,
                          kind="ExternalInput")
    wf1_d = nc.dram_tensor("wf1", [L, FCH, 128, DCH, 128], bf16,
                           kind="ExternalInput")
    wf2_d = nc.dram_tensor("wf2", [L, DCH, 128, FCH, 128], bf16,
                           kind="ExternalInput")
    biasp_d = nc.dram_tensor("biasp", [128, L, 32], f32, kind="ExternalInput")
    qkbp_d = nc.dram_tensor("qkbp", [128, L, 2 * DCH], f32,
                            kind="ExternalInput")
    jones_d = nc.dram_tensor("jones", [128, 128], bf16, kind="ExternalInput")
    jones1_d = nc.dram_tensor("jones1", [128, 128], bf16,
                              kind="ExternalInput")
    jones32_d = nc.dram_tensor("jones32", [128, 128], f32,
                               kind="ExternalInput")
    mask01_d = nc.dram_tensor("mask01", [128, 2, M], bf16,
                              kind="ExternalInput")
    mask01t_d = nc.dram_tensor("mask01t", [128, 2, M4], bf16,
                               kind="ExternalInput")
    hw1_d = nc.dram_tensor("hw1", [H_SLOTS, 2 * DCH, 128, IDH], bf16,
                           kind="ExternalInput")
    hw2_d = nc.dram_tensor("hw2", [H_SLOTS, 4, 128, IDH], bf16,
                           kind="ExternalInput")
    hw3p_d = nc.dram_tensor("hw3p", [128, H_SLOTS * 4], bf16,
                            kind="ExternalInput")
    hb1p_d = nc.dram_tensor("hb1p", [128, H_SLOTS, 4], f32,
                            kind="ExternalInput")
    hb2p_d = nc.dram_tensor("hb2p", [128, H_SLOTS, 4], f32,
                            kind="ExternalInput")
    hb3p_d = nc.dram_tensor("hb3p", [1, H_SLOTS], f32, kind="ExternalInput")

    out_d = nc.dram_tensor("out_h", [1, H_SLOTS, B], f32,
                           kind="ExternalOutput")

    with tile.TileContext(nc) as tc:
        from contextlib import ExitStack
        with ExitStack() as ctx:
            const = ctx.enter_context(tc.tile_pool(name="const", bufs=1))
            ps_a = ctx.enter_context(
                tc.tile_pool(name="ps_a", bufs=7, space="PSUM"))
            ps_ln = ctx.enter_context(
                tc.tile_pool(name="ps_ln", bufs=1, space="PSUM"))
            dram = ctx.enter_context(
                tc.tile_pool(name="dram", bufs=1, space="DRAM"))
            # head pools outlive the transformer pools -> created first
            w1h_pool = ctx.enter_context(tc.tile_pool(name="w1h", bufs=3))
            w2h_pool = ctx.enter_context(tc.tile_pool(name="w2h", bufs=2))
            hact = ctx.enter_context(tc.tile_pool(name="hact", bufs=2))
            tfs = ctx.enter_context(ExitStack())
            hres = tfs.enter_context(tc.tile_pool(name="hres", bufs=1))
            sq = tfs.enter_context(tc.tile_pool(name="sq", bufs=3))
            stats = tfs.enter_context(tc.tile_pool(name="stats", bufs=5))
            actb = tfs.enter_context(tc.tile_pool(name="actb", bufs=2))
            qkp = tfs.enter_context(tc.tile_pool(name="qkp", bufs=1))
            vtokp = tfs.enter_context(tc.tile_pool(name="vtokp", bufs=1))
            attp = tfs.enter_context(tc.tile_pool(name="attp", bufs=6))
            obufp = tfs.enter_context(tc.tile_pool(name="obufp", bufs=1))
            g1p = tfs.enter_context(tc.tile_pool(name="g1p", bufs=1))
            wqk_pool = tfs.enter_context(tc.tile_pool(name="wqk", bufs=6))
            wv_pool = tfs.enter_context(tc.tile_pool(name="wv", bufs=8))
            wo_pool = tfs.enter_context(tc.tile_pool(name="wo", bufs=3))
            wf1_pool = tfs.enter_context(tc.tile_pool(name="wf1", bufs=6))
            wf2_pool = tfs.enter_context(tc.tile_pool(name="wf2", bufs=3))

            # constants
            jones = const.tile([128, 128], bf16, tag="jones")
            nc.sync.dma_start(jones[:], jones_d[:])
            jones1 = const.tile([128, 128], bf16, tag="jones1")
            nc.sync.dma_start(jones1[:], jones1_d[:])
            jones32 = const.tile([128, 128], f32, tag="jones32")
            nc.sync.dma_start(jones32[:], jones32_d[:])
            mask01 = const.tile([128, 2, M], bf16, tag="mask01")
            nc.sync.dma_start(mask01[:], mask01_d[:])
            mask01t = const.tile([128, 2, M4], bf16, tag="mask01t")
            nc.sync.dma_start(mask01t[:], mask01t_d[:])
            qkbp = const.tile([128, L, 2 * DCH], f32, tag="qkbp")
            nc.sync.dma_start(qkbp[:], qkbp_d[:])
            biasp = const.tile([128, L, 32], f32, tag="biasp")
            nc.sync.dma_start(biasp[:], biasp_d[:])
            # LN epsilons as [128,1] const APs (float act-bias needs an AP)
            eps0 = const.tile([128, 1], f32, tag="eps0")
            nc.vector.memset(eps0[:], 1e-6)
            eps1 = const.tile([128, 1], f32, tag="eps1")
            nc.vector.memset(eps1[:], 1e-5)

            # residual, feature-major [128p, chunk, token], fp32
            h = hres.tile([128, DCH, M], f32, tag="h")
            for c in range(DCH):
                nc.sync.dma_start(h[:, c, :], x_d[c])

            w1h_tiles, w2h_tiles = [], []

            def warm(n):
                """Keep-warm dummy matmuls: hold the PE busy (and its DVFS
                clock up) while the LN stats chain runs on DVE/ACT."""
                if n <= 0:
                    return
                ps_w = ps_a.tile([128, 2, M], f32, tag="a", name=None)
                for wi in range(n):
                    nc.tensor.matmul(ps_w[:, wi % 2, :], jones[:],
                                     mask01[:, 0, :],
                                     start=(wi < 2), stop=(wi >= n - 2))

            def layer_norm(l, site, src, mcols, mtag):
                """LN over the feature (partition*chunk) axis.

                src: [128, DCH, mcols] fp32 tile (or view).
                Returns xln [128, DCH, mcols] bf16.
                site 0: collapsed double-LN -> rsqrt(v + 1e-6), host folds
                the 1/sqrt(1+1e-5) into g.  site 1: rsqrt(v + 1e-5).
                alpha = Exp(-0.5 * Ln(v + eps)) (stays in the exp table set).
                """
                eps = eps0 if site == 0 else eps1
                # ps_sq[:,0,:] = mean via direct fp32 ones-matmul (4 cyc/row
                # but zero cross-engine serialization); ps_sq[:,1,:] = E[h^2]
                # via ACT Square -> bf16 matmul (Square is in every act
                # table set). jones/jones32 are pre-scaled by 1/D on host.
                ps_sq = ps_ln.tile([128, 2, mcols], f32, tag="ln")
                for c in range(DCH):
                    nc.tensor.matmul(ps_sq[:, 0, :], jones32[:],
                                     src[:, c, :],
                                     start=(c == 0), stop=(c == DCH - 1))
                for c in range(DCH):
                    hsq = sq.tile([128, mcols], bf16, tag="sq" + mtag)
                    nc.scalar.activation(hsq[:], src[:, c, :], Act.Square)
                    nc.tensor.matmul(ps_sq[:, 1, :], jones[:], hsq[:],
                                     start=(c == 0), stop=(c == DCH - 1))
                warm(N_WARM)
                mu_t = stats.tile([128, mcols], f32, tag="st" + mtag)
                nc.vector.tensor_copy(mu_t[:], ps_sq[:, 0, :])
                mu = mu_t[:]
                mu2 = stats.tile([128, mcols], f32, tag="st" + mtag)
                nc.vector.tensor_mul(mu2[:], mu, mu)
                v = stats.tile([128, mcols], f32, tag="st" + mtag)
                nc.vector.tensor_sub(v[:], ps_sq[:, 1, :], mu2[:])
                lnv = stats.tile([128, mcols], f32, tag="st" + mtag)
                nc.scalar.activation(lnv[:], v[:], Act.Ln, bias=eps[:])
                alpha = stats.tile([128, mcols], f32, tag="st" + mtag)
                nc.scalar.activation(alpha[:], lnv[:], Act.Exp, scale=-0.5)
                xln = actb.tile([128, DCH, mcols], bf16, tag="xln" + mtag)
                for cs in ((0, 2), (2, 4), (4, 6), (6, 8)):
                    c0, c1 = cs
                    w = c1 - c0
                    tt_full = sq.tile([128, 2, mcols], f32, tag="sqf" + mtag)
                    t = tt_full[:, :w, :]
                    mu_b = mu.unsqueeze(1).to_broadcast([128, w, mcols])
                    al_b = alpha[:].unsqueeze(1).to_broadcast([128, w, mcols])
                    nc.vector.tensor_sub(t[:], src[:, c0:c1, :], mu_b)
                    nc.vector.tensor_mul(xln[:, c0:c1, :], t[:], al_b)
                return xln

            for l in range(L):
                last = (l == L - 1)
                # number of query-token columns this layer actually needs
                mq = M4 if last else M
                msk = mask01t if last else mask01

                # ---- attn pre-LN (collapsed double LN)
                xln = layer_norm(l, 0, h[:], M, "")

                # t01 columns of xln for layer-4 Q, packed [128, DCH, 16]
                if last:
                    xln_q = actb.tile([128, DCH, M4], bf16, tag="xlnq")
                    nc.gpsimd.tensor_copy(
                        xln_q[:],
                        xln[:].rearrange("p c (b t) -> p c b t",
                                         b=B_LOC)[:, :, :, 0:2])

                # ---- Q,K feature-major GEMM -> qk [128, 2*DCH, M]
                # (layer 4: Q only at t01 columns, stored in qk[:, n, 0:16])
                qk = qkp.tile([128, 2 * DCH, M], bf16, tag="qk")
                for np_ in range(DCH):
                    ps = ps_a.tile([128, 2, M], f32, tag="a")
                    for i in range(2):
                        n = 2 * np_ + i            # Q chunk index 0..7 x2
                        wt = wqk_pool.tile([128, DCH, 128], bf16, tag="wqk")
                        nc.sync.dma_start(wt[:], wqk_d[l, n])
                        if last and n < DCH:
                            for c in range(DCH):
                                nc.tensor.matmul(ps[:, i, 0:M4],
                                                 wt[:, c, :], xln_q[:, c, :],
                                                 start=(c == 0),
                                                 stop=(c == DCH - 1))
                        else:
                            for c in range(DCH):
                                nc.tensor.matmul(ps[:, i, :],
                                                 wt[:, c, :], xln[:, c, :],
                                                 start=(c == 0),
                                                 stop=(c == DCH - 1))
                    for i in range(2):
                        n = 2 * np_ + i
                        w_cols = M4 if (last and n < DCH) else M
                        nc.scalar.activation(qk[:, n, 0:w_cols],
                                             ps[:, i, 0:w_cols],
                                             Act.Identity,
                                             bias=qkbp[:, l, n:n + 1])

                # ---- V token-major GEMM (activation-stationary):
                # vtok [128 tok, 2, INNER]
                vtok = vtokp.tile([128, 2, INNER], bf16, tag="vtok")
                wv_keep = []
                for c in range(DCH):
                    wvt = wv_pool.tile([128, INNER], bf16, tag="wv")
                    nc.sync.dma_start(wvt[:], wv_d[l, c])
                    wv_keep.append(wvt)
                for mc in range(2):
                    for jb in range(2):
                        psv = ps_a.tile([128, 512], f32, tag="a")
                        for c in range(DCH):
                            nc.tensor.matmul(
                                psv[:],
                                xln[:, c, mc * 128:(mc + 1) * 128],
                                wv_keep[c][:, jb * 512:(jb + 1) * 512],
                                start=(c == 0), stop=(c == DCH - 1))
                        nc.scalar.activation(
                            vtok[:, mc, jb * 512:(jb + 1) * 512], psv[:],
                            Act.Copy)

                # ---- attention, head-pair by head-pair
                # (layer 4: only mq=16 query columns)
                obuf = obufp.tile([128, DCH, M], bf16, tag="obuf")
                for hc in range(DCH):
                    # head pair (2*hc, 2*hc+1): even at partitions 0:64,
                    # odd at 64:128 -> S matmuls interleave across row
                    # groups so the PE array runs them concurrently.
                    es = []
                    pss = []
                    for j in range(2):
                        e_j = attp.tile([128, 2, mq], bf16, tag="e",
                                        name=f"e_{l}_{hc}_{j}")
                        ps_j = ps_a.tile([128, 2, mq], f32, tag="a",
                                         name=f"pss_{l}_{hc}_{j}")
                        es.append(e_j)
                        pss.append(ps_j)
                    for i in range(2):
                        for j in range(2):
                            hp = j * 64
                            nc.tensor.matmul(
                                pss[j][:, i, :],
                                qk[hp:hp + 64, DCH + hc,
                                   i * 128:(i + 1) * 128],
                                qk[hp:hp + 64, hc, 0:mq],
                                start=True, stop=True)
                    for j in range(2):
                        nc.scalar.activation(es[j][:], pss[j][:], Act.Exp,
                                             scale=0.125)
                    # zero cross-batch pairs (block-diag mask) on GPSIMD
                    for j in range(2):
                        nc.gpsimd.tensor_mul(es[j][:], es[j][:], msk[:])
                    for j in range(2):
                        hh = 2 * hc + j
                        hp = j * 64
                        e = es[j]
                        ps_dn = ps_a.tile([128, mq], f32, tag="a")
                        for i in range(2):
                            nc.tensor.matmul(ps_dn[:], jones1[:], e[:, i, :],
                                             start=(i == 0), stop=(i == 1))
                        rd = stats.tile([128, mq], f32, tag="strd")
                        nc.vector.reciprocal_approx_fast(rd[0:64, :],
                                                         ps_dn[0:64, :])
                        ps_o = ps_a.tile([128, mq], f32, tag="a")
                        for i in range(2):
                            nc.tensor.matmul(
                                ps_o[hp:hp + 64, :],
                                vtok[:, i, hh * 64:(hh + 1) * 64],
                                e[:, i, :],
                                start=(i == 0), stop=(i == 1),
                                tile_position=(0, hp))
                        nc.vector.tensor_tensor(
                            obuf[hp:hp + 64, hc, 0:mq], ps_o[hp:hp + 64, :],
                            rd[0:64, :], Alu.mult)

                # ---- output projection (+ residual + out_b)
                # layer 4: only t01 columns, into h_mid (no h update needed)
                if last:
                    h_mid = hres.tile([128, DCH, M4], f32, tag="hmid")
                    h_t01 = h[:].rearrange("p c (b t) -> p c b t",
                                           b=B_LOC)[:, :, :, 0:2]
                for n in range(DCH):
                    wot = wo_pool.tile([128, DCH, 128], bf16, tag="wo")
                    nc.sync.dma_start(wot[:], wo_d[l, n])
                    ps = ps_a.tile([128, mq], f32, tag="a")
                    for c in range(DCH):
                        nc.tensor.matmul(ps[:],
                                         wot[:, c, :],
                                         obuf[:, c, 0:mq],
                                         start=(c == 0), stop=(c == DCH - 1))
                    if last:
                        nc.vector.scalar_tensor_tensor(
                            h_mid[:, n, :], ps[:], biasp[:, l, n:n + 1],
                            h_t01[:, n], Alu.add, Alu.add)
                    else:
                        nc.vector.scalar_tensor_tensor(
                            h[:, n, :], ps[:], biasp[:, l, n:n + 1],
                            h[:, n, :], Alu.add, Alu.add)

                # ---- ff pre-LN
                if last:
                    xln2 = layer_norm(l, 1, h_mid[:], M4, "t")
                else:
                    xln2 = layer_norm(l, 1, h[:], M, "")

                # ---- ff1 + gelu(x + b1) -> g1 [128, FCH, mq]
                g1 = g1p.tile([128, FCH, M], bf16, tag="g1")
                for n in range(FCH):
                    wft = wf1_pool.tile([128, DCH, 128], bf16, tag="wf1")
                    nc.sync.dma_start(wft[:], wf1_d[l, n])
                    ps = ps_a.tile([128, mq], f32, tag="a")
                    for c in range(DCH):
                        nc.tensor.matmul(ps[:],
                                         wft[:, c, :],
                                         xln2[:, c, :],
                                         start=(c == 0), stop=(c == DCH - 1))
                    nc.scalar.activation(g1[:, n, 0:mq], ps[:], Act.Gelu,
                                         bias=biasp[:, l, 8 + n:9 + n])

                # ---- ff2 (+ residual + b2); layer 4 writes x16 instead
                if last:
                    x16 = const.tile([128, DCH, 2, B_LOC], bf16, tag="x16")
                    hm_v = h_mid[:].rearrange("p c (b t) -> p c b t", b=B_LOC)
                for n in range(DCH):
                    wft = wf2_pool.tile([128, FCH, 128], bf16, tag="wf2")
                    nc.sync.dma_start(wft[:], wf2_d[l, n])
                    ps = ps_a.tile([128, mq], f32, tag="a")
                    for c in range(FCH):
                        nc.tensor.matmul(ps[:],
                                         wft[:, c, :],
                                         g1[:, c, 0:mq],
                                         start=(c == 0), stop=(c == FCH - 1))
                    if last:
                        # x16[:, n, tt, b] = ps[(b,t)] + b2 + h_mid[(b,t)]
                        nc.vector.scalar_tensor_tensor(
                            x16[:, n, :, :].rearrange("p tt b -> p b tt"),
                            ps[:], biasp[:, l, 24 + n:25 + n],
                            hm_v[:, n], Alu.add, Alu.add)
                    else:
                        nc.vector.scalar_tensor_tensor(
                            h[:, n, :], ps[:], biasp[:, l, 24 + n:25 + n],
                            h[:, n, :], Alu.add, Alu.add)

                # prefetch head weights during layers 2 and 3 (2 slots each)
                if l in (1, 2):
                    for sn in (2 * (l - 1), 2 * (l - 1) + 1):
                        w1h = w1h_pool.tile([128, 2 * DCH, IDH], bf16,
                                            tag="w1h")
                        nc.sync.dma_start(
                            w1h[:], hw1_d[sn].rearrange("c p n2 -> p c n2"))
                        w2h = w2h_pool.tile([128, 4, IDH], bf16, tag="w2h")
                        nc.sync.dma_start(
                            w2h[:], hw2_d[sn].rearrange("c p n2 -> p c n2"))
                        w1h_tiles.append(w1h)
                        w2h_tiles.append(w2h)

            # ---------------- AllGather x_comb ----------------
            tfs.close()   # free transformer pools for the heads stage
            cc_in = dram.tile([128, 128], bf16)
            nc.sync.dma_start(cc_in[:],
                              x16[:].rearrange("p c tt b -> p (c tt b)"))
            cc_out = dram.tile([N_CORES * 128, 128], bf16)
            nc.gpsimd.collective_compute(
                "AllGather", Alu.bypass,
                replica_groups=[list(range(N_CORES))],
                ins=[cc_in[:].opt()], outs=[cc_out[:].opt()])

            gsb = const.tile([128, N_CORES, 128], bf16, tag="gsb")
            nc.sync.dma_start(
                gsb[:], cc_out[:].rearrange("(j p) f -> p j f", p=128))
            # PE warm-up after the AllGather idle gap
            ps_w = ps_a.tile([128, 2, M], f32, tag="a", name="warmup_ps")
            for wi in range(12):
                nc.tensor.matmul(ps_w[:, wi % 2, :], jones[:],
                                 mask01[:, 0, :],
                                 start=(wi < 2), stop=(wi >= 10))

            # ---------------- 26 (padded 32) MLP heads, feature-major ------
            hb1p = const.tile([128, H_SLOTS, 4], f32, tag="hb1p")
            nc.sync.dma_start(hb1p[:], hb1p_d[:])
            hb2p = const.tile([128, H_SLOTS, 4], f32, tag="hb2p")
            nc.sync.dma_start(hb2p[:], hb2p_d[:])
            hb3p = const.tile([1, H_SLOTS], f32, tag="hb3p")
            nc.sync.dma_start(hb3p[:], hb3p_d[:])
            hw3 = const.tile([128, H_SLOTS * 4], bf16, tag="hw3")
            nc.sync.dma_start(hw3[:], hw3p_d[:])
            outacc = const.tile([1, H_SLOTS, B], f32, tag="outacc")

            # x_comb^T [2D, B] packed: kc = tt*DCH + c, batch = j*8 + b
            # (gsb free layout per core j is (c, tt, b)).
            xcombT = const.tile([128, 2 * DCH, B], bf16, tag="xcombT")
            for kc in range(2 * DCH):
                tt, c = kc // DCH, kc % DCH
                nc.vector.tensor_copy(
                    xcombT[:, kc, :],
                    gsb[:, :, c * 16 + tt * 8: c * 16 + tt * 8 + 8])

            def xcomb_rhs(kc):
                return xcombT[:, kc, :]

            for n in range(H_SLOTS):
                w1h = w1h_tiles[n]
                w2h = w2h_tiles[n]
                # h1^T [512, 64] feature-major, 4 idh chunks
                ps1 = ps_a.tile([128, 4, B], f32, tag="a")
                for nc2 in range(4):
                    for kc in range(2 * DCH):
                        nc.tensor.matmul(
                            ps1[:, nc2, :],
                            w1h[:, kc, nc2 * 128:(nc2 + 1) * 128],
                            xcomb_rhs(kc),
                            start=(kc == 0), stop=(kc == 2 * DCH - 1))
                h1t = hact.tile([128, 4, B], bf16, tag="h1t")
                for nc2 in range(4):
                    nc.scalar.activation(h1t[:, nc2, :], ps1[:, nc2, :],
                                         Act.Relu,
                                         bias=hb1p[:, n, nc2:nc2 + 1])
                ps2 = ps_a.tile([128, 4, B], f32, tag="a")
                for nc2 in range(4):
                    for kc in range(4):
                        nc.tensor.matmul(
                            ps2[:, nc2, :],
                            w2h[:, kc, nc2 * 128:(nc2 + 1) * 128],
                            h1t[:, kc, :],
                            start=(kc == 0), stop=(kc == 3))
                h2t = hact.tile([128, 4, B], bf16, tag="h2t")
                for nc2 in range(4):
                    nc.scalar.activation(h2t[:, nc2, :], ps2[:, nc2, :],
                                         Act.Relu,
                                         bias=hb2p[:, n, nc2:nc2 + 1])
                ps3 = ps_a.tile([1, B], f32, tag="a")
                for kc in range(4):
                    nc.tensor.matmul(ps3[:],
                                     hw3[:, n * 4 + kc:n * 4 + kc + 1],
                                     h2t[:, kc, :],
                                     start=(kc == 0), stop=(kc == 3))
                nc.vector.tensor_scalar_add(outacc[:, n, :], ps3[:],
                                            hb3p[0:1, n:n + 1])

            nc.sync.dma_start(out_d[:], outacc[:])

    nc.finalize()
    return nc


# ---------------------------------------------------------------- host side

def _prep_in_maps(inputs):
    x = np.asarray(inputs['x'], np.float32)
    qkv_w = np.asarray(inputs['qkv_w'], np.float32)
    out_w = np.asarray(inputs['out_w'], np.float32)
    out_b = np.asarray(inputs['out_b'], np.float32)
    attn_ln_g = np.asarray(inputs['attn_ln_g'], np.float32)
    attn_ln_b = np.asarray(inputs['attn_ln_b'], np.float32)
    ff_ln_g = np.asarray(inputs['ff_ln_g'], np.float32)
    ff_ln_b = np.asarray(inputs['ff_ln_b'], np.float32)
    ff_w1 = np.asarray(inputs['ff_w1'], np.float32)
    ff_b1 = np.asarray(inputs['ff_b1'], np.float32)
    ff_w2 = np.asarray(inputs['ff_w2'], np.float32)
    ff_b2 = np.asarray(inputs['ff_b2'], np.float32)
    head_w1 = np.asarray(inputs['head_w1'], np.float32)
    head_b1 = np.asarray(inputs['head_b1'], np.float32)
    head_w2 = np.asarray(inputs['head_w2'], np.float32)
    head_b2 = np.asarray(inputs['head_b2'], np.float32)
    head_w3 = np.asarray(inputs['head_w3'], np.float32)
    head_b3 = np.asarray(inputs['head_b3'], np.float32)

    # Fold the LN affine transform into the following GEMM weights (exact):
    #   xln = (h-mu)*alpha_hat;  y = (xln*g + b) @ W = xln @ (diag(g) W) + b@W
    # The collapsed double-LN 1/sqrt(1+1e-5) factor is folded into g too.
    ag_eff = attn_ln_g * np.float32((1.0 + 1e-5) ** -0.5)   # [L, D]
    qkvb = np.einsum('ld,ldn->ln', attn_ln_b, qkv_w)        # [L, 3*INNER]
    ff_b1 = ff_b1 + np.einsum('ld,ldn->ln', ff_ln_b, ff_w1)
    qkv_w = qkv_w * ag_eff[:, :, None]
    ff_w1 = ff_w1 * ff_ln_g[:, :, None]
    # V's LN-bias contribution passes through softmax unchanged (weights sum
    # to 1), so it folds into the output-projection bias exactly.
    vbias = qkvb[:, 2 * INNER:]                              # [L, INNER]
    out_b = out_b + np.einsum('lk,lkd->ld', vbias, out_w)

    # shared (replicated) weight tensors in per-chunk stream layouts
    wqk = np.zeros((L, 16, 128, DCH, 128), np.float32)
    wv = np.zeros((L, DCH, 128, INNER), np.float32)
    wo = np.zeros((L, DCH, 128, DCH, 128), np.float32)
    wf1 = np.zeros((L, FCH, 128, DCH, 128), np.float32)
    wf2 = np.zeros((L, DCH, 128, FCH, 128), np.float32)
    for l in range(L):
        wqk[l] = qkv_w[l][:, :2 * INNER].reshape(
            DCH, 128, 16, 128).transpose(2, 1, 0, 3)
        wv[l] = qkv_w[l][:, 2 * INNER:].reshape(DCH, 128, INNER)
        wo[l] = out_w[l].reshape(DCH, 128, DCH, 128).transpose(2, 1, 0, 3)
        wf1[l] = ff_w1[l].reshape(DCH, 128, FCH, 128).transpose(2, 1, 0, 3)
        wf2[l] = ff_w2[l].reshape(FCH, 128, DCH, 128).transpose(2, 1, 0, 3)
    wqk = np.ascontiguousarray(wqk).astype(BF16)
    wv = np.ascontiguousarray(wv).astype(BF16)
    wo = np.ascontiguousarray(wo).astype(BF16)
    wf1 = np.ascontiguousarray(wf1).astype(BF16)
    wf2 = np.ascontiguousarray(wf2).astype(BF16)

    biasp = np.zeros((128, L, 32), np.float32)
    biasp[:, :, 0:8] = out_b.reshape(L, 8, 128).transpose(2, 0, 1)
    biasp[:, :, 8:24] = ff_b1.reshape(L, 16, 128).transpose(2, 0, 1)
    biasp[:, :, 24:32] = ff_b2.reshape(L, 8, 128).transpose(2, 0, 1)

    qkbp = np.ascontiguousarray(
        qkvb[:, :2 * INNER].reshape(L, 2 * DCH, 128).transpose(2, 0, 1))

    jones = np.full((128, 128), 1.0 / D, np.float32).astype(BF16)
    jones1 = np.ones((128, 128), np.float32).astype(BF16)
    jones32 = np.full((128, 128), 1.0 / D, np.float32)

    # 0/1 block-diagonal attention mask: key token k = i*128 + p (batch
    # k//32), query token m (batch m//32); full M and t01-only variants.
    mask01 = np.zeros((128, 2, M), np.float32)
    for i in range(2):
        for p in range(128):
            kb = (i * 128 + p) // T
            mask01[p, i, kb * T:(kb + 1) * T] = 1.0
    mask01t = np.zeros((128, 2, M4), np.float32)
    for i in range(2):
        for p in range(128):
            kb = (i * 128 + p) // T
            mask01t[p, i, kb * 2:(kb + 1) * 2] = 1.0
    mask01 = mask01.astype(BF16)
    mask01t = mask01t.astype(BF16)

    in_maps = []
    for c in range(N_CORES):
        xs = x[c * B_LOC:(c + 1) * B_LOC].reshape(M, D)  # [256, 1024]
        x_fm = np.ascontiguousarray(xs.T.reshape(DCH, 128, M))

        hw1 = np.zeros((H_SLOTS, 2 * DCH, 128, IDH), np.float32)
        hw2 = np.zeros((H_SLOTS, 4, 128, IDH), np.float32)
        hw3p = np.zeros((128, H_SLOTS * 4), np.float32)
        hb1p = np.zeros((128, H_SLOTS, 4), np.float32)
        hb2p = np.zeros((128, H_SLOTS, 4), np.float32)
        hb3p = np.zeros((1, H_SLOTS), np.float32)
        for n in range(H_SLOTS):
            g = n * N_CORES + c
            if g >= NOUT:
                continue
            hw1[n] = head_w1[g].reshape(2 * DCH, 128, IDH)
            hw2[n] = head_w2[g].reshape(4, 128, IDH)
            hw3p[:, n * 4:(n + 1) * 4] = head_w3[g].reshape(4, 128).T
            hb1p[:, n, :] = head_b1[g].reshape(4, 128).T
            hb2p[:, n, :] = head_b2[g].reshape(4, 128).T
            hb3p[0, n] = head_b3[g, 0]
        in_maps.append({
            'x': x_fm,
            'wqk': wqk, 'wv': wv, 'wo': wo, 'wf1': wf1, 'wf2': wf2,
            'biasp': biasp, 'qkbp': qkbp,
            'jones': jones, 'jones1': jones1, 'jones32': jones32,
            'mask01': mask01, 'mask01t': mask01t,
            'hw1': hw1.astype(BF16), 'hw2': hw2.astype(BF16),
            'hw3p': hw3p.astype(BF16),
            'hb1p': hb1p, 'hb2p': hb2p, 'hb3p': hb3p,
        })
    return in_maps


def _get_nc():
    if 'nc' not in _CACHE:
        _CACHE['nc'] = _build_nc()
    return _CACHE['nc']


def _unshard_out(results):
    out = np.zeros((B, NOUT, 1), np.float32)
    for c in range(N_CORES):
        oh = results[c]['out_h']           # [1, H_SLOTS, B]
        for n in range(H_SLOTS):
            g = n * N_CORES + c
            if g < NOUT:
                out[:, g, 0] = oh[0, n, :]
    return out


def kernel(**inputs):
    from concourse.bass_utils import run_bass_kernel_spmd
    nc = _get_nc()
    in_maps = _prep_in_maps(inputs)
    res = run_bass_kernel_spmd(nc, in_maps, core_ids=list(range(N_CORES)))
    return _unshard_out(res.results)
